# revision 1
# baseline (speedup 1.0000x reference)
"""Bass/Trainium2 kernel for nn_Attention_46566035423948.

Multi-head attention (B=4, N=2048, C=1024, H=16) on 8 NeuronCores.
Sharding: core c = (batch b = c//2, head-group g = c%2, 8 heads each).
Each core computes a partial projection output [N, C]; the host sums the
two head-group partials per batch and adds b_proj.

Per-core dataflow (everything in "key/channel-on-partition" layout so the
softmax denominator is a matmul reduction):
  phase 1: Q^T,K^T [512, 2048] fp32r (head pairs packed 64+64 in partition
           chunks), V [2048, 8*65] natural layout bf16 with a ones column
           per head, from bf16 xT and the W_qkv slices (SCALE pre-folded
           into Wq on host).
  phase 2: per (q-block 512, head-pair, k-chunk 128):
           S^T = K^T.T @ Q^T (row-tiled pair of fp32r matmuls)
           P^T = exp(S^T) * maskT (ScalarE exp PSUM->SBUF bf16, DVE mask)
           U  += V_aug.T @ P^T (M=65: row 64 accumulates the softmax
           denominators for free), then xn^T = U * broadcast(1/U[64]).
           Phase 1 is interleaved pair-by-pair under q-blocks 0-1 so the
           ScalarE exp pipeline (the throughput floor) starts early.
  phase 3: out = xn^T.T @ W_proj_slice (fp32r), staged through SBUF to
           DRAM. Proj groups for completed q-blocks are streamed into the
           last head-pair's k-loops; only the final q-block's groups run
           as a tail.
"""

import numpy as np
import ml_dtypes

import concourse.mybir as mybir
import concourse.tile as tile
from concourse import bacc
from concourse import bass_utils

N_CORES = 8
B, N, C, H = 4, 2048, 1024, 16
HS = C // H           # 64
SCALE = HS ** -0.5
HPC = 8               # heads per core
GW = HPC * HS         # 512: per-core head-group width
PAIRS = 4             # head pairs per core
CC = C // 128         # 8 contraction chunks over C
KC = N // 128         # 16 key chunks
QB = N // 512         # 4 query blocks of 512
QC = N // 128         # 16 query chunks of 128 (proj)

F32 = mybir.dt.float32
F32R = mybir.dt.float32r
BF16 = mybir.dt.bfloat16
EXP = mybir.ActivationFunctionType.Exp

_NC_CACHE = []


def _load_mask(nc, m_pool, mT, qb):
    """DMA the 16 [128, 512] bf16 mask tiles for one q-block."""
    tiles = []
    for kc in range(KC):
        mt = m_pool.tile([128, 512], BF16, name="m_t", tag="m_t")
        nc.sync.dma_start(mt[:], mT[qb, kc])
        tiles.append(mt)
    return tiles


def _phase2_block(nc, qb, pair, qkt, v_t, xn, mtiles,
                  s_pool, u_pool, e_pool, p_pool,
                  rinv_pool, b_pool, pre_kc=None):
    """Attention for one (q-block, head-pair): S^T, exp, mask, augmented PV
    (which also accumulates the softmax denominators in U row 64), then
    normalize into xn[pair][:, qb*512:(qb+1)*512]."""
    qs = slice(qb * 512, (qb + 1) * 512)
    KT = qkt[("k", pair)]
    QT = qkt[("q", pair)]
    U = u_pool.tile([128, 1024], F32, name="U", tag="U")
    h0 = 2 * pair * (HS + 1)
    h1 = (2 * pair + 1) * (HS + 1)
    SKEW = 5  # PV trails S by 5 k-chunks so the first S's of a unit issue
    # before PV(kc=0) blocks the PE stream on the previous unit's normalize
    p_tiles = {}

    def emit_pv(kc):
        P = p_tiles.pop(kc)
        nc.tensor.matmul(
            U[0:65, 0:512], v_t[kc][:, h0:h0 + 65],
            P[:, 0:512], start=(kc == 0), stop=(kc == KC - 1),
            tile_position=(0, 0), skip_group_check=True)
        nc.tensor.matmul(
            U[0:65, 512:1024], v_t[kc][:, h1:h1 + 65],
            P[:, 512:1024], start=(kc == 0), stop=(kc == KC - 1),
            tile_position=(0, 0), skip_group_check=True)

    for kc in range(KC + SKEW):
        if kc < KC:
            if pre_kc is not None:
                pre_kc(kc)
            ks = slice(kc * 128, (kc + 1) * 128)
            mt = mtiles[kc]
            S = s_pool.tile([128, 1024], F32, name="S", tag="S")
            nc.tensor.matmul(S[:, 0:512], KT[0:64, ks], QT[0:64, qs],
                             start=True, stop=True, tile_position=(0, 0))
            nc.tensor.matmul(S[:, 512:1024], KT[64:128, ks], QT[64:128, qs],
                             start=True, stop=True, tile_position=(64, 0))
            E = e_pool.tile([128, 1024], BF16, name="E", tag="E")
            nc.scalar.activation(E[:], S[:], EXP)
            P = p_pool.tile([128, 1024], BF16, name="P", tag="P")
            nc.vector.tensor_mul(P[:, 0:512], E[:, 0:512], mt[:])
            nc.vector.tensor_mul(P[:, 512:1024], E[:, 512:1024], mt[:])
            p_tiles[kc] = P
        if kc >= SKEW:
            emit_pv(kc - SKEW)
    r01 = rinv_pool.tile([1, 1024], F32, name="r01", tag="r01")
    nc.vector.reciprocal(r01[:], U[64:65, :])
    Bc = b_pool.tile([128, 1024], F32, name="Bc", tag="Bc")
    nc.gpsimd.partition_broadcast(Bc[:], r01[:])
    nc.vector.tensor_mul(xn[pair][0:64, qs], U[0:64, 0:512], Bc[0:64, 0:512])
    nc.vector.tensor_mul(xn[pair][64:128, qs], U[0:64, 512:1024],
                         Bc[0:64, 512:1024])


def _emit(tc, xT, wq, wk, wv, mT, wp, out):
    nc = tc.nc
    from contextlib import ExitStack

    with ExitStack() as stack:
        # persistent pools: V lives through phase 2, xn through phase 3
        v_pool = stack.enter_context(tc.tile_pool(name="vp", bufs=KC))
        xn_pool = stack.enter_context(tc.tile_pool(name="xn", bufs=1))
        wp_pool = stack.enter_context(tc.tile_pool(name="wpp", bufs=PAIRS))
        ostage_pool = stack.enter_context(tc.tile_pool(name="ostage", bufs=6))

        v_t = []
        xn = [xn_pool.tile([128, N], F32R, name=f"xn{i}", tag=f"xn{i}")
              for i in range(PAIRS)]
        wp_t = []

        # single fused region: QKV production for (pair, qb) is interleaved
        # directly ahead of the attention block that consumes it, so the
        # ScalarE exp stream starts almost immediately and PE fills ACT
        # stalls with projection work throughout.
        with tc.tile_pool(name="qkt", bufs=4) as qkt_pool, \
             tc.tile_pool(name="ep", bufs=4) as e_pool, \
             tc.tile_pool(name="pp", bufs=7) as p_pool, \
             tc.tile_pool(name="rinv", bufs=2) as rinv_pool, \
             tc.tile_pool(name="binv", bufs=2) as b_pool, \
             tc.tile_pool(name="xt", bufs=1) as xt_pool, \
             tc.tile_pool(name="wqk", bufs=4) as wqk_pool, \
             tc.tile_pool(name="wvp", bufs=1) as wv_pool, \
             tc.tile_pool(name="mp", bufs=8) as m_pool, \
             tc.tile_pool(name="ps2s", bufs=2, space="PSUM") as s_pool, \
             tc.tile_pool(name="ps1", bufs=2, space="PSUM") as ps1_pool, \
             tc.tile_pool(name="ps2u", bufs=1, space="PSUM") as u_pool:

            def dma_wqk(pair):
                wts = {}
                for which, wsrc in (("k", wk), ("q", wq)):
                    wt = wqk_pool.tile([128, CC * 128], BF16, name="wqk_t",
                                       tag="wqk_t")
                    nc.sync.dma_start(wt[:], wsrc[pair])
                    for cc in range(CC):
                        wts[(which, cc)] = wt[:, cc * 128:(cc + 1) * 128]
                return wts

            # DMA in PE-consumption order: pair-0 K weights, the 8 qb=0
            # xt chunks and wv first — the first K-group starts after
            # ~0.75 MB of DMA.
            def dma_w(which, wsrc, pair, wts):
                wt = wqk_pool.tile([128, CC * 128], BF16, name="wqk_t",
                                   tag="wqk_t")
                nc.sync.dma_start(wt[:], wsrc[pair])
                for cc in range(CC):
                    wts[(which, cc)] = wt[:, cc * 128:(cc + 1) * 128]

            wts0 = {}
            dma_w("k", wk, 0, wts0)
            xt_q = {}
            wv_t = []
            t = xt_pool.tile([128, CC * 512], BF16, name="xt_0")
            half = CC * 256
            nc.sync.dma_start(t[:, 0:half], xT[0, :, 0:half])
            nc.sync.dma_start(t[:, half:], xT[0, :, half:])
            for cc in range(CC):
                xt_q[(cc, 0)] = t[:, cc * 512:(cc + 1) * 512]
            dma_w("q", wq, 0, wts0)
            t = wv_pool.tile([128, CC * 512], BF16, name="wv_all")
            nc.sync.dma_start(t[:, 0:half], wv[:, 0:half])
            nc.sync.dma_start(t[:, half:], wv[:, half:])
            for cc in range(CC):
                wv_t.append(t[:, cc * 512:(cc + 1) * 512])
            for qb in range(1, QB):
                t = xt_pool.tile([128, CC * 512], BF16, name=f"xt_{qb}")
                nc.sync.dma_start(t[:], xT[qb])
                for cc in range(CC):
                    xt_q[(cc, qb)] = t[:, cc * 512:(cc + 1) * 512]

            # V tiles are emitted lazily inside the first attention block's
            # k-loop so ScalarE's exp pipeline starts early. Layout
            # [128, 8*65]: head h at cols h*65..h*65+64 plus a ones column
            # at h*65+64, so PV matmuls (M=65) also produce the softmax
            # row sums.
            def emit_v(kc):
                ps = ps1_pool.tile([128, 512], F32, name="ps1t", tag="ps1t")
                for cc in range(CC):
                    nc.tensor.matmul(
                        ps[:],
                        xt_q[(cc, kc // 4)][:, (kc % 4) * 128:
                                            (kc % 4) * 128 + 128],
                        wv_t[cc],
                        start=(cc == 0), stop=(cc == CC - 1))
                t = v_pool.tile([128, HPC * (HS + 1)], BF16, name="v_t",
                                tag="v_t")
                tv = t[:].rearrange("p (h d) -> p h d", h=HPC)
                nc.gpsimd.memset(tv[:, :, HS:HS + 1], 1.0)
                nc.vector.tensor_copy(
                    tv[:, :, 0:HS],
                    ps[:].rearrange("p (h d) -> p h d", h=HPC))
                v_t.append(t)

            proj_done = []

            def proj_group(qc, nh, pool=None):
                pool = pool if pool is not None else ps1_pool
                ps = pool.tile([128, 512], F32, name="ps1t", tag="ps1t")
                for pair_ in range(PAIRS):
                    nc.tensor.matmul(
                        ps[:], xn[pair_][:, qc * 128:(qc + 1) * 128],
                        wp_t[pair_][:, nh * 512:(nh + 1) * 512],
                        start=(pair_ == 0), stop=(pair_ == PAIRS - 1))
                ost = ostage_pool.tile([128, 512], F32, name="ost", tag="ost")
                nc.scalar.copy(ost[:], ps[:])
                nc.sync.dma_start(
                    out[qc * 128:(qc + 1) * 128, nh * 512:(nh + 1) * 512],
                    ost[:])
                proj_done.append((qc, nh))

            for pair in range(PAIRS):
                wts = wts0 if pair == 0 else dma_wqk(pair)
                qkt = {}
                for which in ("q", "k"):
                    qkt[(which, pair)] = qkt_pool.tile(
                        [128, N], F32R, name="qkt_t", tag="qkt_t")
                if pair == 1:
                    # prefetch proj weights once SBUF headroom exists
                    for pp_ in range(PAIRS):
                        t = wp_pool.tile([128, C], F32R, name="wp_t",
                                         tag="wp_t")
                        nc.sync.dma_start(
                            t[:], wp[pp_ * 128:(pp_ + 1) * 128, :])
                        wp_t.append(t)
                def qk_group(which, qb):
                    dst = qkt[(which, pair)]
                    ps = ps1_pool.tile([128, 512], F32, name="ps1t",
                                       tag="ps1t")
                    for cc in range(CC):
                        nc.tensor.matmul(
                            ps[:], wts[(which, cc)][:],
                            xt_q[(cc, qb)][:],
                            start=(cc == 0), stop=(cc == CC - 1))
                    nc.vector.tensor_copy(
                        dst[:, qb * 512:(qb + 1) * 512], ps[:])

                # K^T is contracted over ALL key blocks by every attention
                # block, so it must be complete before (or produced just
                # ahead of) the k-chunks that read it. For pair 0 the later
                # K-groups are injected into the first block's k-loop (with
                # V) so the exp stream starts after ~2 QK groups, not 5.
                if pair == 0:
                    qk_group("k", 0)

                    def pre0(kc):
                        if kc in (1, 5, 9):
                            qk_group("k", kc // 4 + 1)
                        emit_v(kc)
                else:
                    for qb in range(QB):
                        qk_group("k", qb)
                    pre0 = None
                for qb in range(QB):
                    qk_group("q", qb)
                    if pair == 0 and qb == 0:
                        pre = pre0
                    elif pair == PAIRS - 1 and qb >= 1:
                        # last pair: the q-blocks processed so far have
                        # complete xn across all pairs — stream their proj
                        # groups into this unit's k-loop (PE slack fills
                        # while ACT stays the critical engine)
                        done = set(proj_done)
                        lim, step = (8, 2) if qb == QB - 1 else (5, 3)
                        pend = [(c, n) for c in range(qb * 4)
                                for n in range(2) if (c, n) not in done][:lim]

                        def pre(kc, _p=pend, _s=step):
                            if _p and kc % _s == 1:
                                proj_group(*_p.pop(0))
                    else:
                        pre = None
                    mtiles = _load_mask(nc, m_pool, mT, qb)
                    _phase2_block(nc, qb, pair, qkt, v_t, xn, mtiles,
                                  s_pool, u_pool,
                                  e_pool, p_pool, rinv_pool, b_pool,
                                  pre_kc=pre)

            # remaining proj groups (qb3's q-chunks + any not streamed)
            done = set(proj_done)
            for qc in range(QC):
                for nh in range(2):
                    if (qc, nh) not in done:
                        proj_group(qc, nh)


def build():
    if _NC_CACHE:
        return _NC_CACHE[0]
    nc = bacc.Bacc("TRN2", target_bir_lowering=False, debug=False,
                   enable_asserts=False, num_devices=N_CORES)
    xT = nc.dram_tensor("xT", [QB, 128, CC * 512], BF16,
                        kind="ExternalInput").ap()
    wq = nc.dram_tensor("wq", [PAIRS, 128, CC * 128], BF16,
                        kind="ExternalInput").ap()
    wk = nc.dram_tensor("wk", [PAIRS, 128, CC * 128], BF16,
                        kind="ExternalInput").ap()
    wv = nc.dram_tensor("wv", [128, CC * 512], BF16,
                        kind="ExternalInput").ap()
    mT = nc.dram_tensor("mT", [QB, KC, 128, 512], BF16,
                        kind="ExternalInput").ap()
    wp = nc.dram_tensor("wp", [GW, C], F32R, kind="ExternalInput").ap()
    out = nc.dram_tensor("out", [N, C], F32, kind="ExternalOutput").ap()
    with tile.TileContext(nc) as tc:
        _emit(tc, xT, wq, wk, wv, mT, wp, out)
    nc.compile()
    _NC_CACHE.append(nc)
    return nc


def _tile4(a, rows, cols):
    """[R, Q] -> [Q//cols, R//rows, rows, cols] contiguous tiles so every
    device DMA is a single contiguous transfer."""
    R, Q = a.shape
    return np.ascontiguousarray(
        a.reshape(R // rows, rows, Q // cols, cols).transpose(0, 2, 1, 3)
         .transpose(1, 0, 2, 3))


def _pack_cc(a, cols):
    """[C, Q] -> [Q//cols, 128, (C//128)*cols]: per q-block, the 8
    contraction chunks side by side on 128 partitions (one contiguous DMA
    per q-block)."""
    R, Q = a.shape
    t = a.reshape(R // 128, 128, Q // cols, cols)      # [cc, p, qb, c]
    return np.ascontiguousarray(
        t.transpose(2, 1, 0, 3).reshape(Q // cols, 128, (R // 128) * cols))


def shard_inputs(joint_feature, mask, W_qkv, W_proj, b_proj):
    mT = _tile4(np.ascontiguousarray(mask[0, 0].T).astype(ml_dtypes.bfloat16),
                128, 512)
    in_maps = []
    for c in range(N_CORES):
        b, g = divmod(c, 2)
        lo, hi = g * GW, (g + 1) * GW
        in_maps.append({
            "xT": _pack_cc(np.ascontiguousarray(joint_feature[b].T)
                           .astype(ml_dtypes.bfloat16), 512),
            "wq": _pack_cc((W_qkv[:, lo:hi] * SCALE)
                           .astype(ml_dtypes.bfloat16), 128),
            "wk": _pack_cc(W_qkv[:, C + lo:C + hi]
                           .astype(ml_dtypes.bfloat16), 128),
            "wv": _pack_cc(W_qkv[:, 2 * C + lo:2 * C + hi]
                           .astype(ml_dtypes.bfloat16), 512)[0],
            "mT": mT,
            "wp": np.ascontiguousarray(W_proj[lo:hi, :]).astype(np.float32),
        })
    return in_maps


def kernel(joint_feature, mask, W_qkv, W_proj, b_proj):
    joint_feature = np.asarray(joint_feature, dtype=np.float32)
    mask = np.asarray(mask)
    W_qkv = np.asarray(W_qkv, dtype=np.float32)
    W_proj = np.asarray(W_proj, dtype=np.float32)
    b_proj = np.asarray(b_proj, dtype=np.float32)

    nc = build()
    in_maps = shard_inputs(joint_feature, mask, W_qkv, W_proj, b_proj)
    res = bass_utils.run_bass_kernel_spmd(nc, in_maps,
                                          core_ids=list(range(N_CORES)))
    out = np.empty((B, N, C), dtype=np.float32)
    for b in range(B):
        out[b] = res.results[2 * b]["out"] + res.results[2 * b + 1]["out"] \
            + b_proj
    return out



# revision 36
# speedup vs baseline: 1.0684x; 1.0684x over previous
"""Bass/Trainium2 kernel for nn_Attention_46566035423948.

Multi-head attention (B=4, N=2048, C=1024, H=16) on 8 NeuronCores.
Sharding: core c = (batch b = c//2, head-group g = c%2, 8 heads each).
Each core computes a partial projection output [N, C]; the host sums the
two head-group partials per batch and adds b_proj.

Per-core dataflow (key-on-partition for S/exp, query-on-partition for PV):
  phase 1: Q^T,K^T [512, 2048] bf16 (head pairs packed 64+64 in partition
           chunks), V [2048, 2*65] per pair (ones column per head; SCALE
           pre-folded into Wq on host), from bf16 xT and W_qkv slices.
           V and K production for pair p is spread across pair p-1's
           k-loop slots so no single block is PE-overloaded.
  phase 2: per block = (head-pair, q-block 512), k-loop over 16 k-chunks:
           S^T = K^T.T @ Q^T (row-tiled pair of matmuls)
           P = exp(S^T) * mask (ScalarE exp PSUM->SBUF bf16, then one DVE
           mul in place with the mask broadcast across the two heads).
           All 16 P tiles of a block are kept; the block's PV runs
           group-by-group streamed into the NEXT block's k-loop: per
           (head, q-chunk 128) region, 16 consecutive matmuls with P as
           the stationary operand and V_aug (65 cols incl. ones) moving:
           U[q, 65] += P_chunk.T @ V_aug. Full 128 output partitions and
           65 streamed rows per matmul (vs 512 with V stationary); col 64
           accumulates the softmax denominator. Regions are consecutive
           because PSUM allows one open accumulation group per bank.
           Normalize: per-partition reciprocal of U[:,64] +
           tensor_scalar_mul into xn_q [128 q, 512] bf16; a DMA-engine
           xbar transpose flips each (pair, q-block) to xnT [128 dims, N]
           so PE spends no rows transposing.
  phase 3: out = xnT.T @ W_proj_slice (bf16), staged through SBUF (GpSimd
           copy) to DRAM. Proj groups for completed q-blocks are streamed
           into the last head-pair's k-loops after the PV-stream slots;
           the final q-block's PV + proj interleave per q-chunk in the
           tail (per-chunk transposes).
"""

import numpy as np
import ml_dtypes

import concourse.mybir as mybir
import concourse.tile as tile
from concourse import bacc
from concourse import bass_utils
from concourse.masks import make_identity

N_CORES = 8
B, N, C, H = 4, 2048, 1024, 16
HS = C // H           # 64
SCALE = HS ** -0.5
HPC = 8               # heads per core
GW = HPC * HS         # 512: per-core head-group width
PAIRS = 4             # head pairs per core
CC = C // 128         # 8 contraction chunks over C
KC = N // 128         # 16 key chunks
QB = N // 512         # 4 query blocks of 512
QC = N // 128         # 16 query chunks of 128 (proj)

F32 = mybir.dt.float32
BF16 = mybir.dt.bfloat16
EXP = mybir.ActivationFunctionType.Exp

_NC_CACHE = []


def _emit(tc, xT, wq, wk, wv, mT, wp, out):
    nc = tc.nc
    from contextlib import ExitStack

    with ExitStack() as stack:
        # persistent pools: V lives through phase 2, xnT through phase 3
        v_pool = stack.enter_context(tc.tile_pool(name="vp", bufs=PAIRS * KC))
        xn_pool = stack.enter_context(tc.tile_pool(name="xn", bufs=1))
        wp_pool = stack.enter_context(tc.tile_pool(name="wpp", bufs=PAIRS))
        ostage_pool = stack.enter_context(tc.tile_pool(name="ostage", bufs=8))

        v_t = {}
        xnT = [xn_pool.tile([128, N], BF16, name=f"xn{i}", tag=f"xn{i}")
               for i in range(PAIRS)]
        ident = xn_pool.tile([128, 128], BF16, name="ident", tag="ident")
        wp_t = []

        with tc.tile_pool(name="qkt", bufs=4) as qkt_pool, \
             tc.tile_pool(name="ep", bufs=28) as e_pool, \
             tc.tile_pool(name="rinv", bufs=4) as rinv_pool, \
             tc.tile_pool(name="xnq", bufs=2) as xnq_pool, \
             tc.tile_pool(name="xt", bufs=1) as xt_pool, \
             tc.tile_pool(name="wqk", bufs=8) as wqk_pool, \
             tc.tile_pool(name="wvp", bufs=1) as wv_pool, \
             tc.tile_pool(name="mp", bufs=3) as m_pool, \
             tc.tile_pool(name="ps2s", bufs=2, space="PSUM") as s_pool, \
             tc.tile_pool(name="ps1", bufs=2, space="PSUM") as ps1_pool, \
             tc.tile_pool(name="ps2u", bufs=1, space="PSUM") as u_pool:

            make_identity(nc, ident[:])

            # warm the PE clock during the input-DMA wait: the p-state
            # model halves matmul throughput until ~3us of sustained
            # execution, which would otherwise tax the first QK groups
            warm = xnq_pool.tile([128, 512], BF16, name="warm", tag="xn_q")
            nc.gpsimd.memset(warm[:], 0.0)
            wps = ps1_pool.tile([128, 512], F32, name="ps1t", tag="ps1t")
            for i in range(8):
                nc.tensor.matmul(wps[:], warm[:, 0:128], warm[:],
                                 start=(i == 0), stop=(i == 7))

            # --- input DMA, in PE-consumption order ----------------------
            wts = {}           # (which, pair, cc) -> weight slice

            def dma_w(which, wsrc, pair):
                wt = wqk_pool.tile([128, CC * 128], BF16, name="wqk_t",
                                   tag="wqk_t")
                nc.sync.dma_start(wt[:], wsrc[pair])
                for cc in range(CC):
                    wts[(which, pair, cc)] = wt[:, cc * 128:(cc + 1) * 128]

            dma_w("k", wk, 0)
            xt_q = {}
            wv_t = []
            t = xt_pool.tile([128, CC * 512], BF16, name="xt_0")
            half = CC * 256
            nc.sync.dma_start(t[:, 0:half], xT[0, :, 0:half])
            nc.sync.dma_start(t[:, half:], xT[0, :, half:])
            for cc in range(CC):
                xt_q[(cc, 0)] = t[:, cc * 512:(cc + 1) * 512]
            dma_w("q", wq, 0)
            t = wv_pool.tile([128, CC * 512], BF16, name="wv_all")
            nc.sync.dma_start(t[:, 0:half], wv[:, 0:half])
            nc.sync.dma_start(t[:, half:], wv[:, half:])
            for cc in range(CC):
                wv_t.append(t[:, cc * 512:(cc + 1) * 512])
            for qb in range(1, QB):
                t = xt_pool.tile([128, CC * 512], BF16, name=f"xt_{qb}")
                nc.sync.dma_start(t[:], xT[qb])
                for cc in range(CC):
                    xt_q[(cc, qb)] = t[:, cc * 512:(cc + 1) * 512]
            for p in range(1, PAIRS):
                dma_w("k", wk, p)
                dma_w("q", wq, p)

            # --- building blocks -----------------------------------------
            def emit_v(pair, kc):
                """V for (pair, k-chunk): [128 pos, 2*65] bf16 with ones
                columns (PV moving operand)."""
                ps = ps1_pool.tile([128, 128], F32, name="psv", tag="ps1t")
                for cc in range(CC):
                    nc.tensor.matmul(
                        ps[:],
                        xt_q[(cc, kc // 4)][:, (kc % 4) * 128:
                                            (kc % 4) * 128 + 128],
                        wv_t[cc][:, pair * 128:(pair + 1) * 128],
                        start=(cc == 0), stop=(cc == CC - 1))
                t = v_pool.tile([128, 130], BF16, name="v_t", tag="v_t")
                tv = t[:].rearrange("p (h d) -> p h d", h=2)
                nc.gpsimd.memset(tv[:, :, HS:HS + 1], 1.0)
                nc.vector.tensor_copy(
                    tv[:, :, 0:HS],
                    ps[:].rearrange("p (h d) -> p h d", h=2))
                v_t[(pair, kc)] = t

            qkt = {}

            def qk_group(which, pair, qb):
                dst = qkt[(which, pair)]
                ps = ps1_pool.tile([128, 512], F32, name="ps1t", tag="ps1t")
                for cc in range(CC):
                    nc.tensor.matmul(
                        ps[:], wts[(which, pair, cc)][:],
                        xt_q[(cc, qb)][:],
                        start=(cc == 0), stop=(cc == CC - 1))
                nc.vector.tensor_copy(
                    dst[:, qb * 512:(qb + 1) * 512], ps[:])

            proj_done = []
            ost_tiles = {}     # (qb, nh, half) -> (tile, count)
            deferred_dmas = []

            def flush_out_dmas():
                while deferred_dmas:
                    dst, ost = deferred_dmas.pop(0)
                    nc.sync.dma_start(dst, ost)

            def proj_group(qc, nh, pool=None, defer=False):
                if pool is None:
                    ps = ps1_pool.tile([128, 512], F32, name="ps1t",
                                       tag="ps1t")
                else:
                    # tail: rotate through the (now idle) S ring so proj
                    # groups pipeline 4 deep without extra PSUM
                    ps = pool.tile([128, 512], F32, name="S", tag="S")
                for pair_ in range(PAIRS):
                    nc.tensor.matmul(
                        ps[:], xnT[pair_][:, qc * 128:(qc + 1) * 128],
                        wp_t[pair_][:, nh * 512:(nh + 1) * 512],
                        start=(pair_ == 0), stop=(pair_ == PAIRS - 1))
                qb_, qcw_ = divmod(qc, 4)
                key = (qb_, nh, qcw_ // 2)
                if key not in ost_tiles:
                    ost_tiles[key] = [ostage_pool.tile(
                        [128, 1024], BF16, name="ost", tag="ost"), 0]
                ost, cnt = ost_tiles[key]
                if nh == 0:
                    nc.scalar.copy(
                        ost[:, (qcw_ % 2) * 512:(qcw_ % 2 + 1) * 512], ps[:])
                else:
                    nc.vector.tensor_copy(
                        ost[:, (qcw_ % 2) * 512:(qcw_ % 2 + 1) * 512], ps[:])
                ost_tiles[key][1] = cnt + 1
                if cnt + 1 == 2:
                    # one batched output DMA per (q-block, C-half, qc-pair);
                    # in the last pair's k-loops the DMA emission is
                    # deferred so its copy-wait can't park on SP ahead of
                    # the next block's transposes
                    r0 = qb_ * 512 + (qcw_ // 2) * 256
                    dst = out[r0:r0 + 256, nh * 512:(nh + 1) * 512] \
                        .rearrange("(qc p) q -> p qc q", p=128)
                    srcv = ost[:].rearrange("p (qc q) -> p qc q", qc=2)
                    if defer:
                        deferred_dmas.append((dst, srcv))
                    else:
                        nc.sync.dma_start(dst, srcv)
                proj_done.append((qc, nh))

            # --- deferred PV machinery ------------------------------------
            def pv_mms(st, r, kcs):
                """Accumulation matmuls for one (head, q-chunk) region.
                All of a region's matmuls form one PSUM group (one open
                group per bank), but may be emitted in separate runs."""
                pair, qb, p_tiles, U, xn_q = st
                head, qcw = divmod(r, 4)
                for kc in kcs:
                    nc.tensor.matmul(
                        U[:, r * 128:r * 128 + 65],
                        p_tiles[kc][:, head * 512 + qcw * 128:
                                    head * 512 + qcw * 128 + 128],
                        v_t[(pair, kc)][:, head * 65:head * 65 + 65],
                        start=(kc == 0), stop=(kc == KC - 1),
                        tile_position=(0, 0), skip_group_check=True)

            def pv_norm(st, r):
                pair, qb, p_tiles, U, xn_q = st
                head, qcw = divmod(r, 4)
                rinv = rinv_pool.tile([128, 1], F32, name="rinv", tag="rinv")
                nc.vector.reciprocal(rinv[:], U[:, r * 128 + 64:r * 128 + 65])
                nc.vector.tensor_scalar_mul(
                    xn_q[:, qcw * 128 + head * 64:qcw * 128 + head * 64 + 64],
                    U[:, r * 128:r * 128 + 64], rinv[:])

            def pv_region(st, r):
                pv_mms(st, r, range(KC))
                pv_norm(st, r)

            def pe_transpose(st, qcw):
                """Last-pair transpose via the PE array (PSUM-local, ~0.6us
                chain) instead of the ~3us DMA-xbar path."""
                pair, qb, p_tiles, U, xn_q = st
                pt = ps1_pool.tile([128, 128], BF16, name="ptt", tag="ps1t")
                nc.tensor.transpose(
                    pt[:], xn_q[:, qcw * 128:(qcw + 1) * 128], ident[:])
                nc.vector.tensor_copy(
                    xnT[pair][:, qb * 512 + qcw * 128:
                              qb * 512 + (qcw + 1) * 128], pt[:])

            def pv_transpose(st, qcw=None):
                pair, qb, p_tiles, U, xn_q = st
                if qcw is None:
                    nc.sync.dma_start_transpose(
                        xnT[pair][:, qb * 512:(qb + 1) * 512]
                        .rearrange("p (four q) -> p four q", four=4),
                        xn_q[:])
                else:
                    nc.sync.dma_start_transpose(
                        xnT[pair][:, qb * 512 + qcw * 128:
                                  qb * 512 + (qcw + 1) * 128],
                        xn_q[:, qcw * 128:(qcw + 1) * 128])

            def block_kloop(pair, qb, prev_st, slot_hooks):
                """S/exp/mask k-loop for one block; streams prev block's PV
                regions (slots 3..10) and its transposes."""
                U = u_pool.tile([128, 1024], F32, name="U", tag="U")
                xn_q = xnq_pool.tile([128, 512], BF16, name="xn_q",
                                     tag="xn_q")
                qs = slice(qb * 512, (qb + 1) * 512)
                KT = qkt[("k", pair)]
                QT = qkt[("q", pair)]
                mtiles = []

                def load_mask(g):
                    mt = m_pool.tile([128, 2048], BF16, name="m_t",
                                     tag="m_t")
                    nc.sync.dma_start(mt[:], mT[qb, g])
                    for i in range(4):
                        mtiles.append(mt[:, i * 512:(i + 1) * 512])

                load_mask(0)
                load_mask(1)
                p_tiles = []
                for kc in range(KC):
                    if kc == 2:
                        load_mask(2)
                    elif kc == 6:
                        load_mask(3)
                    # S + exp + mask first: the exp stream is the global
                    # pacer, so the next S must never queue behind a slot's
                    # hook lump on the in-order PE
                    ks = slice(kc * 128, (kc + 1) * 128)
                    S = s_pool.tile([128, 1024], F32, name="S", tag="S")
                    nc.tensor.matmul(S[:, 0:512], KT[0:64, ks], QT[0:64, qs],
                                     start=True, stop=True,
                                     tile_position=(0, 0))
                    nc.tensor.matmul(S[:, 512:1024], KT[64:128, ks],
                                     QT[64:128, qs],
                                     start=True, stop=True,
                                     tile_position=(64, 0))
                    E = e_pool.tile([128, 1024], BF16, name="E", tag="E")
                    nc.scalar.activation(E[:], S[:], EXP)
                    ev = E[:].rearrange("p (two q) -> p two q", two=2)
                    # last pair: DVE also carries the proj staging copies,
                    # so shift a quarter of the mask-mults to idle GpSimd
                    eng = nc.gpsimd if pair == PAIRS - 1 and kc % 8 == 0 \
                        else nc.vector
                    eng.tensor_mul(
                        ev, ev, mtiles[kc][:, None, :]
                        .broadcast_to((128, 2, 512)))
                    p_tiles.append(E)
                    if prev_st is not None:
                        # regions start at slot 3: P(kc15) of the previous
                        # block is only ready ~2 slots after the boundary,
                        # and a region's final accumulation matmul would
                        # park the in-order PE queue until it lands.
                        # Paired order (a q-chunk's two heads adjacent) so
                        # each q-chunk's transpose can issue early.
                        if 3 <= kc <= 10:
                            qcw_, h_ = divmod(kc - 3, 2)
                            pv_region(prev_st, h_ * 4 + qcw_)
                            if h_ == 1 and prev_st[0] == PAIRS - 1:
                                # prev block is in the last pair: immediate
                                # per-q-chunk PE transpose so its proj can
                                # stream this block with a short chain
                                pe_transpose(prev_st, qcw_)
                        if prev_st[0] != PAIRS - 1 and kc == 11:
                            pv_transpose(prev_st)
                    for fn in slot_hooks.get(kc, ()):
                        fn()
                    if pair == PAIRS - 1 and qb == QB - 1 and kc >= 12:
                        # pre-run the tail's first two regions (banks 0/1)
                        # against the P tiles already produced; PSUM is
                        # free of the prev block's U after slot 10
                        st_self = (pair, qb, p_tiles, U, xn_q)
                        if kc == 12:
                            pv_mms(st_self, 0, range(0, 10))
                        elif kc == 13:
                            pv_mms(st_self, 4, range(0, 10))
                        elif kc == 14:
                            pv_mms(st_self, 0, (10, 11))
                            pv_mms(st_self, 4, (10, 11))
                        else:
                            pv_mms(st_self, 0, (12,))
                            pv_mms(st_self, 4, (12,))
                return (pair, qb, p_tiles, U, xn_q)

            # --- drive the 16 blocks --------------------------------------
            prev_st = None
            qkt[("k", 0)] = qkt_pool.tile([128, N], BF16, name="qkt_t",
                                          tag="qkt_t")
            qkt[("q", 0)] = qkt_pool.tile([128, N], BF16, name="qkt_t",
                                          tag="qkt_t")
            for pair in range(PAIRS):
                if pair < PAIRS - 1:
                    qkt[("k", pair + 1)] = qkt_pool.tile(
                        [128, N], BF16, name="qkt_t", tag="qkt_t")
                    qkt[("q", pair + 1)] = qkt_pool.tile(
                        [128, N], BF16, name="qkt_t", tag="qkt_t")
                if pair == 1:
                    # prefetch proj weights once SBUF headroom exists
                    for pp_ in range(PAIRS):
                        t = wp_pool.tile([128, C], BF16, name="wp_t",
                                         tag="wp_t")
                        nc.sync.dma_start(
                            t[:], wp[pp_ * 128:(pp_ + 1) * 128, :])
                        wp_t.append(t)

                if pair == 0:
                    qk_group("k", 0, 0)
                    qk_group("q", 0, 0)
                for qb in range(QB):
                    hooks = {}
                    # host the NEXT block's Q-group at slot 13 so block
                    # boundaries carry no serial PE lump
                    bi = pair * QB + qb
                    if bi + 1 < PAIRS * QB:
                        npair, nqb = divmod(bi + 1, QB)
                        hooks.setdefault(14, []).append(
                            lambda p=npair, q=nqb: qk_group("q", p, q))
                    if pair == 0 and qb == 0:
                        # pair0: V chunks + remaining K-blocks just-in-time
                        # (last 4 V chunks spill into the next block's
                        # early slots: consumed there from slot 3 on)
                        for kc in range(KC):
                            if kc in (1, 5, 9):
                                kb = kc // 4 + 1
                                hooks.setdefault(kc, []).append(
                                    lambda kb=kb: qk_group("k", 0, kb))
                            if kc < 12:
                                hooks.setdefault(kc, []).append(
                                    lambda kc=kc: emit_v(0, kc))
                    else:
                        if pair == 0 and qb == 1:
                            for i, kc in enumerate(range(12, 16)):
                                hooks.setdefault(i // 2, []).append(
                                    lambda kc=kc: emit_v(0, kc))
                        # spread next pair's V (16 chunks) and K (4 groups)
                        # across slots 9..12 and 15 of qb1..3 blocks
                        if pair < PAIRS - 1 and qb >= 1:
                            np_ = pair + 1
                            hooks.setdefault(0, []).append(
                                lambda p=np_, kb=qb - 1:
                                qk_group("k", p, kb))
                            if qb == 3:
                                hooks.setdefault(15, []).append(
                                    lambda p=np_: qk_group("k", p, 3))
                                vcs = range(12, 16)
                            else:
                                vcs = range((qb - 1) * 6, qb * 6)
                            for i, kc in enumerate(vcs):
                                hooks.setdefault(1 + i // 2, []).append(
                                    lambda p=np_, kc=kc: emit_v(p, kc))
                        if pair == PAIRS - 1 and qb >= 1:
                            # stream prev q-block's proj as its per-chunk
                            # transposes (slots 6..9) land
                            slots = (6, 7, 8, 9, 10, 11, 12, 13)
                            groups = [((qb - 1) * 4 + qcw, nh)
                                      for qcw in range(4) for nh in range(2)]
                            for s, (c, n) in zip(slots, groups):
                                hooks.setdefault(s, []).append(
                                    lambda c=c, n=n:
                                    proj_group(c, n, defer=True))
                            hooks.setdefault(14, []).insert(
                                0, flush_out_dmas)
                    prev_st = block_kloop(pair, qb, prev_st, hooks)

            # --- tail: last block's PV software-pipelined with its proj ---
            tailpools = (None, s_pool)
            ti = 0

            def tail_proj(qcw):
                nonlocal ti
                qc = (QB - 1) * 4 + qcw
                for nh in range(2):
                    proj_group(qc, nh, pool=tailpools[ti % 2])
                    ti += 1

            pv_mms(prev_st, 0, (13, 14, 15))
            pv_norm(prev_st, 0)
            pv_mms(prev_st, 4, (13, 14, 15))
            pv_norm(prev_st, 4)
            pe_transpose(prev_st, 0)
            pv_region(prev_st, 1)
            pv_region(prev_st, 5)
            pe_transpose(prev_st, 1)
            flush_out_dmas()
            tail_proj(0)
            pv_region(prev_st, 2)
            pv_region(prev_st, 6)
            pe_transpose(prev_st, 2)
            tail_proj(1)
            pv_region(prev_st, 3)
            pv_region(prev_st, 7)
            pe_transpose(prev_st, 3)
            tail_proj(2)
            tail_proj(3)

            # safety net: any group not emitted above
            done = set(proj_done)
            for qc in range(QC):
                for nh in range(2):
                    if (qc, nh) not in done:
                        proj_group(qc, nh)


def build():
    if _NC_CACHE:
        return _NC_CACHE[0]
    nc = bacc.Bacc("TRN2", target_bir_lowering=False, debug=False,
                   enable_asserts=False, num_devices=N_CORES)
    xT = nc.dram_tensor("xT", [QB, 128, CC * 512], BF16,
                        kind="ExternalInput").ap()
    wq = nc.dram_tensor("wq", [PAIRS, 128, CC * 128], BF16,
                        kind="ExternalInput").ap()
    wk = nc.dram_tensor("wk", [PAIRS, 128, CC * 128], BF16,
                        kind="ExternalInput").ap()
    wv = nc.dram_tensor("wv", [128, CC * 512], BF16,
                        kind="ExternalInput").ap()
    mT = nc.dram_tensor("mT", [QB, KC // 4, 128, 4 * 512], BF16,
                        kind="ExternalInput").ap()
    wp = nc.dram_tensor("wp", [GW, C], BF16, kind="ExternalInput").ap()
    out = nc.dram_tensor("out", [N, C], BF16, kind="ExternalOutput").ap()
    with tile.TileContext(nc) as tc:
        _emit(tc, xT, wq, wk, wv, mT, wp, out)
    nc.compile()
    _NC_CACHE.append(nc)
    return nc


def _tile4(a, rows, cols):
    """[R, Q] -> [Q//cols, R//rows, rows, cols] contiguous tiles so every
    device DMA is a single contiguous transfer."""
    R, Q = a.shape
    return np.ascontiguousarray(
        a.reshape(R // rows, rows, Q // cols, cols).transpose(0, 2, 1, 3)
         .transpose(1, 0, 2, 3))


def _pack_cc(a, cols):
    """[C, Q] -> [Q//cols, 128, (C//128)*cols]: per q-block, the 8
    contraction chunks side by side on 128 partitions (one contiguous DMA
    per q-block)."""
    R, Q = a.shape
    t = a.reshape(R // 128, 128, Q // cols, cols)      # [cc, p, qb, c]
    return np.ascontiguousarray(
        t.transpose(2, 1, 0, 3).reshape(Q // cols, 128, (R // 128) * cols))


def shard_inputs(joint_feature, mask, W_qkv, W_proj, b_proj):
    mT = _tile4(np.ascontiguousarray(mask[0, 0].T).astype(ml_dtypes.bfloat16),
                128, 512)
    mT = np.ascontiguousarray(
        mT.reshape(QB, KC // 4, 4, 128, 512).transpose(0, 1, 3, 2, 4)
          .reshape(QB, KC // 4, 128, 4 * 512))
    in_maps = []
    for c in range(N_CORES):
        b, g = divmod(c, 2)
        lo, hi = g * GW, (g + 1) * GW
        in_maps.append({
            "xT": _pack_cc(np.ascontiguousarray(joint_feature[b].T)
                           .astype(ml_dtypes.bfloat16), 512),
            "wq": _pack_cc((W_qkv[:, lo:hi] * SCALE)
                           .astype(ml_dtypes.bfloat16), 128),
            "wk": _pack_cc(W_qkv[:, C + lo:C + hi]
                           .astype(ml_dtypes.bfloat16), 128),
            "wv": _pack_cc(W_qkv[:, 2 * C + lo:2 * C + hi]
                           .astype(ml_dtypes.bfloat16), 512)[0],
            "mT": mT,
            "wp": np.ascontiguousarray(W_proj[lo:hi, :])
                  .astype(ml_dtypes.bfloat16),
        })
    return in_maps


def kernel(joint_feature, mask, W_qkv, W_proj, b_proj):
    joint_feature = np.asarray(joint_feature, dtype=np.float32)
    mask = np.asarray(mask)
    W_qkv = np.asarray(W_qkv, dtype=np.float32)
    W_proj = np.asarray(W_proj, dtype=np.float32)
    b_proj = np.asarray(b_proj, dtype=np.float32)

    nc = build()
    in_maps = shard_inputs(joint_feature, mask, W_qkv, W_proj, b_proj)
    res = bass_utils.run_bass_kernel_spmd(nc, in_maps,
                                          core_ids=list(range(N_CORES)))
    out = np.empty((B, N, C), dtype=np.float32)
    for b in range(B):
        out[b] = np.asarray(res.results[2 * b]["out"], dtype=np.float32) \
            + np.asarray(res.results[2 * b + 1]["out"], dtype=np.float32) \
            + b_proj
    return out


# revision 51
# speedup vs baseline: 1.0837x; 1.0143x over previous
"""Bass/Trainium2 kernel for nn_Attention_46566035423948.

Multi-head attention (B=4, N=2048, C=1024, H=16) on 8 NeuronCores.
Sharding: core c = (batch b = c//2, head-group g = c%2, 8 heads each).
Each core computes a partial projection output [N, C]; the host sums the
two head-group partials per batch and adds b_proj.

Per-core dataflow (key-on-partition for S/exp, query-on-partition for PV):
  phase 1: Q^T,K^T [512, 2048] bf16 (head pairs packed 64+64 in partition
           chunks), V [2048, 2*65] per pair (ones column per head; SCALE
           pre-folded into Wq on host), from bf16 xT and W_qkv slices.
           V and K production for pair p is spread across pair p-1's
           k-loop slots so no single block is PE-overloaded.
  phase 2: per block = (head-pair, q-block 512), k-loop over 16 k-chunks:
           S^T = K^T.T @ Q^T (row-tiled pair of matmuls)
           P = exp(S^T) * mask (ScalarE exp PSUM->SBUF bf16, then one DVE
           mul in place with the mask broadcast across the two heads).
           All 16 P tiles of a block are kept; the block's PV runs
           group-by-group streamed into the NEXT block's k-loop: per
           (head, q-chunk 128) region, 16 consecutive matmuls with P as
           the stationary operand and V_aug (65 cols incl. ones) moving:
           U[q, 65] += P_chunk.T @ V_aug. Full 128 output partitions and
           65 streamed rows per matmul (vs 512 with V stationary); col 64
           accumulates the softmax denominator. Regions are consecutive
           because PSUM allows one open accumulation group per bank.
           Normalize: per-partition reciprocal of U[:,64] +
           tensor_scalar_mul into xn_q [128 q, 512] bf16; a DMA-engine
           xbar transpose flips each (pair, q-block) to xnT [128 dims, N]
           so PE spends no rows transposing.
  phase 3: out = xnT.T @ W_proj_slice (bf16), staged through SBUF (GpSimd
           copy) to DRAM. Proj groups for completed q-blocks are streamed
           into the last head-pair's k-loops after the PV-stream slots;
           the final q-block's PV + proj interleave per q-chunk in the
           tail (per-chunk transposes).
"""

import numpy as np
import ml_dtypes

import concourse.mybir as mybir
import concourse.tile as tile
from concourse import bacc
from concourse import bass_utils
from concourse.masks import make_identity

N_CORES = 8
B, N, C, H = 4, 2048, 1024, 16
HS = C // H           # 64
SCALE = HS ** -0.5
HPC = 8               # heads per core
GW = HPC * HS         # 512: per-core head-group width
PAIRS = 4             # head pairs per core
CC = C // 128         # 8 contraction chunks over C
KC = N // 128         # 16 key chunks
QB = N // 512         # 4 query blocks of 512
QC = N // 128         # 16 query chunks of 128 (proj)

F32 = mybir.dt.float32
BF16 = mybir.dt.bfloat16
EXP = mybir.ActivationFunctionType.Exp

_NC_CACHE = []


def _emit(tc, xT, wq, wk, wv, mT, wp, out):
    nc = tc.nc
    from contextlib import ExitStack

    with ExitStack() as stack:
        # persistent pools: V lives through phase 2, xnT through phase 3
        v_pool = stack.enter_context(tc.tile_pool(name="vp", bufs=PAIRS * KC))
        xn_pool = stack.enter_context(tc.tile_pool(name="xn", bufs=1))
        wp_pool = stack.enter_context(tc.tile_pool(name="wpp", bufs=PAIRS))
        ostage_pool = stack.enter_context(tc.tile_pool(name="ostage", bufs=8))

        v_t = {}
        xnT = [xn_pool.tile([128, N], BF16, name=f"xn{i}", tag=f"xn{i}")
               for i in range(PAIRS)]
        ident = xn_pool.tile([128, 128], BF16, name="ident", tag="ident")
        wp_t = []

        with tc.tile_pool(name="qkt", bufs=4) as qkt_pool, \
             tc.tile_pool(name="ep", bufs=28) as e_pool, \
             tc.tile_pool(name="rinv", bufs=4) as rinv_pool, \
             tc.tile_pool(name="xnq", bufs=2) as xnq_pool, \
             tc.tile_pool(name="xt", bufs=1) as xt_pool, \
             tc.tile_pool(name="wqk", bufs=8) as wqk_pool, \
             tc.tile_pool(name="wvp", bufs=1) as wv_pool, \
             tc.tile_pool(name="mp", bufs=3) as m_pool, \
             tc.tile_pool(name="ps2s", bufs=2, space="PSUM") as s_pool, \
             tc.tile_pool(name="ps1", bufs=2, space="PSUM") as ps1_pool, \
             tc.tile_pool(name="ps2u", bufs=1, space="PSUM") as u_pool:

            make_identity(nc, ident[:])

            # warm the PE clock during the input-DMA wait: the p-state
            # model halves matmul throughput until ~3us of sustained
            # execution, which would otherwise tax the first QK groups
            warm = xnq_pool.tile([128, 512], BF16, name="warm", tag="xn_q")
            nc.gpsimd.memset(warm[:], 0.0)
            wps = ps1_pool.tile([128, 512], F32, name="ps1t", tag="ps1t")
            for i in range(8):
                nc.tensor.matmul(wps[:], warm[:, 0:128], warm[:],
                                 start=(i == 0), stop=(i == 7))

            # --- input DMA, in PE-consumption order ----------------------
            wts = {}           # (which, pair, cc) -> weight slice

            def dma_w(which, wsrc, pair):
                wt = wqk_pool.tile([128, CC * 128], BF16, name="wqk_t",
                                   tag="wqk_t")
                nc.sync.dma_start(wt[:], wsrc[pair])
                for cc in range(CC):
                    wts[(which, pair, cc)] = wt[:, cc * 128:(cc + 1) * 128]

            dma_w("k", wk, 0)
            xt_q = {}
            wv_t = []
            t = xt_pool.tile([128, CC * 512], BF16, name="xt_0")
            half = CC * 256
            nc.sync.dma_start(t[:, 0:half], xT[0, :, 0:half])
            nc.sync.dma_start(t[:, half:], xT[0, :, half:])
            for cc in range(CC):
                xt_q[(cc, 0)] = t[:, cc * 512:(cc + 1) * 512]
            dma_w("q", wq, 0)
            t = wv_pool.tile([128, CC * 512], BF16, name="wv_all")
            nc.sync.dma_start(t[:, 0:half], wv[:, 0:half])
            nc.sync.dma_start(t[:, half:], wv[:, half:])
            for cc in range(CC):
                wv_t.append(t[:, cc * 512:(cc + 1) * 512])
            for qb in range(1, QB):
                t = xt_pool.tile([128, CC * 512], BF16, name=f"xt_{qb}")
                nc.sync.dma_start(t[:], xT[qb])
                for cc in range(CC):
                    xt_q[(cc, qb)] = t[:, cc * 512:(cc + 1) * 512]
            for p in range(1, PAIRS):
                dma_w("k", wk, p)
                dma_w("q", wq, p)

            # --- building blocks -----------------------------------------
            def emit_v(pair, kc):
                """V for (pair, k-chunk): [128 pos, 2*65] bf16 with ones
                columns (PV moving operand)."""
                ps = ps1_pool.tile([128, 128], F32, name="psv", tag="ps1t")
                for cc in range(CC):
                    nc.tensor.matmul(
                        ps[:],
                        xt_q[(cc, kc // 4)][:, (kc % 4) * 128:
                                            (kc % 4) * 128 + 128],
                        wv_t[cc][:, pair * 128:(pair + 1) * 128],
                        start=(cc == 0), stop=(cc == CC - 1))
                t = v_pool.tile([128, 130], BF16, name="v_t", tag="v_t")
                tv = t[:].rearrange("p (h d) -> p h d", h=2)
                nc.gpsimd.memset(tv[:, :, HS:HS + 1], 1.0)
                nc.vector.tensor_copy(
                    tv[:, :, 0:HS],
                    ps[:].rearrange("p (h d) -> p h d", h=2))
                v_t[(pair, kc)] = t

            qkt = {}

            def qk_group(which, pair, qb, cols=None):
                dst = qkt[(which, pair)]
                c0, c1 = (0, 512) if cols is None else cols
                w = c1 - c0
                ps = ps1_pool.tile([128, 512], F32, name="ps1t", tag="ps1t")
                for cc in range(CC):
                    nc.tensor.matmul(
                        ps[:, 0:w], wts[(which, pair, cc)][:],
                        xt_q[(cc, qb)][:, c0:c1],
                        start=(cc == 0), stop=(cc == CC - 1))
                nc.vector.tensor_copy(
                    dst[:, qb * 512 + c0:qb * 512 + c1], ps[:, 0:w])

            proj_done = []
            ost_tiles = {}     # (qb, nh, half) -> (tile, count)
            deferred_dmas = []

            def flush_out_dmas():
                while deferred_dmas:
                    dst, ost = deferred_dmas.pop(0)
                    nc.sync.dma_start(dst, ost)

            def proj_group(qc, nh, pool=None, defer=False):
                if pool is None:
                    ps = ps1_pool.tile([128, 512], F32, name="ps1t",
                                       tag="ps1t")
                else:
                    # tail: rotate through the (now idle) S ring so proj
                    # groups pipeline 4 deep without extra PSUM
                    ps = pool.tile([128, 512], F32, name="S", tag="S")
                for pair_ in range(PAIRS):
                    nc.tensor.matmul(
                        ps[:], xnT[pair_][:, qc * 128:(qc + 1) * 128],
                        wp_t[pair_][:, nh * 512:(nh + 1) * 512],
                        start=(pair_ == 0), stop=(pair_ == PAIRS - 1))
                qb_, qcw_ = divmod(qc, 4)
                key = (qb_, nh, qcw_ // 2)
                if key not in ost_tiles:
                    ost_tiles[key] = [ostage_pool.tile(
                        [128, 1024], BF16, name="ost", tag="ost"), 0]
                ost, cnt = ost_tiles[key]
                if nh == 0:
                    nc.scalar.copy(
                        ost[:, (qcw_ % 2) * 512:(qcw_ % 2 + 1) * 512], ps[:])
                else:
                    nc.vector.tensor_copy(
                        ost[:, (qcw_ % 2) * 512:(qcw_ % 2 + 1) * 512], ps[:])
                ost_tiles[key][1] = cnt + 1
                if cnt + 1 == 2:
                    # one batched output DMA per (q-block, C-half, qc-pair);
                    # in the last pair's k-loops the DMA emission is
                    # deferred so its copy-wait can't park on SP ahead of
                    # the next block's transposes
                    r0 = qb_ * 512 + (qcw_ // 2) * 256
                    dst = out[r0:r0 + 256, nh * 512:(nh + 1) * 512] \
                        .rearrange("(qc p) q -> p qc q", p=128)
                    srcv = ost[:].rearrange("p (qc q) -> p qc q", qc=2)
                    if defer:
                        deferred_dmas.append((dst, srcv))
                    else:
                        nc.sync.dma_start(dst, srcv)
                proj_done.append((qc, nh))

            # --- deferred PV machinery ------------------------------------
            def pv_mms(st, r, kcs):
                """Accumulation matmuls for one (head, q-chunk) region.
                All of a region's matmuls form one PSUM group (one open
                group per bank), but may be emitted in separate runs."""
                pair, qb, p_tiles, U, xn_q = st
                head, qcw = divmod(r, 4)
                for kc in kcs:
                    nc.tensor.matmul(
                        U[:, r * 128:r * 128 + 65],
                        p_tiles[kc][:, head * 512 + qcw * 128:
                                    head * 512 + qcw * 128 + 128],
                        v_t[(pair, kc)][:, head * 65:head * 65 + 65],
                        start=(kc == 0), stop=(kc == KC - 1),
                        tile_position=(0, 0), skip_group_check=True)

            def pv_norm(st, r):
                pair, qb, p_tiles, U, xn_q = st
                head, qcw = divmod(r, 4)
                rinv = rinv_pool.tile([128, 1], F32, name="rinv", tag="rinv")
                nc.vector.reciprocal(rinv[:], U[:, r * 128 + 64:r * 128 + 65])
                nc.vector.tensor_scalar_mul(
                    xn_q[:, qcw * 128 + head * 64:qcw * 128 + head * 64 + 64],
                    U[:, r * 128:r * 128 + 64], rinv[:])

            def pv_region(st, r):
                pv_mms(st, r, range(KC))
                pv_norm(st, r)

            def pe_transpose(st, qcw):
                """Last-pair transpose via the PE array (PSUM-local, ~0.6us
                chain) instead of the ~3us DMA-xbar path."""
                pair, qb, p_tiles, U, xn_q = st
                pt = ps1_pool.tile([128, 128], BF16, name="ptt", tag="ps1t")
                nc.tensor.transpose(
                    pt[:], xn_q[:, qcw * 128:(qcw + 1) * 128], ident[:])
                nc.vector.tensor_copy(
                    xnT[pair][:, qb * 512 + qcw * 128:
                              qb * 512 + (qcw + 1) * 128], pt[:])

            def pv_transpose(st, qcw=None):
                pair, qb, p_tiles, U, xn_q = st
                if qcw is None:
                    nc.sync.dma_start_transpose(
                        xnT[pair][:, qb * 512:(qb + 1) * 512]
                        .rearrange("p (four q) -> p four q", four=4),
                        xn_q[:])
                else:
                    nc.sync.dma_start_transpose(
                        xnT[pair][:, qb * 512 + qcw * 128:
                                  qb * 512 + (qcw + 1) * 128],
                        xn_q[:, qcw * 128:(qcw + 1) * 128])

            def block_kloop(pair, qb, prev_st, slot_hooks):
                """S/exp/mask k-loop for one block; streams prev block's PV
                regions (slots 3..10) and its transposes."""
                U = u_pool.tile([128, 1024], F32, name="U", tag="U")
                xn_q = xnq_pool.tile([128, 512], BF16, name="xn_q",
                                     tag="xn_q")
                qs = slice(qb * 512, (qb + 1) * 512)
                KT = qkt[("k", pair)]
                QT = qkt[("q", pair)]
                mtiles = []

                def load_mask(g):
                    mt = m_pool.tile([128, 2048], BF16, name="m_t",
                                     tag="m_t")
                    nc.sync.dma_start(mt[:], mT[qb, g])
                    for i in range(4):
                        mtiles.append(mt[:, i * 512:(i + 1) * 512])

                load_mask(0)
                load_mask(1)
                p_tiles = []
                for kc in range(KC):
                    if kc == 2:
                        load_mask(2)
                    elif kc == 6:
                        load_mask(3)
                    # S + exp + mask first: the exp stream is the global
                    # pacer, so the next S must never queue behind a slot's
                    # hook lump on the in-order PE
                    ks = slice(kc * 128, (kc + 1) * 128)
                    S = s_pool.tile([128, 1024], F32, name="S", tag="S")
                    nc.tensor.matmul(S[:, 0:512], KT[0:64, ks], QT[0:64, qs],
                                     start=True, stop=True,
                                     tile_position=(0, 0))
                    nc.tensor.matmul(S[:, 512:1024], KT[64:128, ks],
                                     QT[64:128, qs],
                                     start=True, stop=True,
                                     tile_position=(64, 0))
                    E = e_pool.tile([128, 1024], BF16, name="E", tag="E")
                    nc.scalar.activation(E[:], S[:], EXP)
                    ev = E[:].rearrange("p (two q) -> p two q", two=2)
                    # last pair: DVE also carries the proj staging copies,
                    # so shift a quarter of the mask-mults to idle GpSimd
                    eng = nc.gpsimd if pair == PAIRS - 1 and kc % 8 == 0 \
                        else nc.vector
                    eng.tensor_mul(
                        ev, ev, mtiles[kc][:, None, :]
                        .broadcast_to((128, 2, 512)))
                    p_tiles.append(E)
                    if prev_st is not None:
                        # regions start at slot 3: P(kc15) of the previous
                        # block is only ready ~2 slots after the boundary,
                        # and a region's final accumulation matmul would
                        # park the in-order PE queue until it lands.
                        # Paired order (a q-chunk's two heads adjacent) so
                        # each q-chunk's transpose can issue early.
                        if 3 <= kc <= 10:
                            qcw_, h_ = divmod(kc - 3, 2)
                            pv_region(prev_st, h_ * 4 + qcw_)
                            if h_ == 1 and prev_st[0] == PAIRS - 1:
                                # prev block is in the last pair: immediate
                                # per-q-chunk PE transpose so its proj can
                                # stream this block with a short chain
                                pe_transpose(prev_st, qcw_)
                        if prev_st[0] != PAIRS - 1 and kc == 11:
                            pv_transpose(prev_st)
                    for fn in slot_hooks.get(kc, ()):
                        fn()
                    if pair == PAIRS - 1 and qb == QB - 1 and kc >= 12:
                        # pre-run the tail's first two regions (banks 0/1)
                        # against the P tiles already produced; PSUM is
                        # free of the prev block's U after slot 10
                        st_self = (pair, qb, p_tiles, U, xn_q)
                        if kc == 12:
                            pv_mms(st_self, 0, range(0, 10))
                        elif kc == 13:
                            pv_mms(st_self, 4, range(0, 10))
                        elif kc == 14:
                            pv_mms(st_self, 0, (10, 11))
                            pv_mms(st_self, 4, (10, 11))
                        else:
                            pv_mms(st_self, 0, (12,))
                            pv_mms(st_self, 4, (12,))
                return (pair, qb, p_tiles, U, xn_q)

            # --- drive the 16 blocks --------------------------------------
            prev_st = None
            qkt[("k", 0)] = qkt_pool.tile([128, N], BF16, name="qkt_t",
                                          tag="qkt_t")
            qkt[("q", 0)] = qkt_pool.tile([128, N], BF16, name="qkt_t",
                                          tag="qkt_t")
            for pair in range(PAIRS):
                if pair < PAIRS - 1:
                    qkt[("k", pair + 1)] = qkt_pool.tile(
                        [128, N], BF16, name="qkt_t", tag="qkt_t")
                    qkt[("q", pair + 1)] = qkt_pool.tile(
                        [128, N], BF16, name="qkt_t", tag="qkt_t")
                if pair == 1:
                    # prefetch proj weights once SBUF headroom exists
                    for pp_ in range(PAIRS):
                        t = wp_pool.tile([128, C], BF16, name="wp_t",
                                         tag="wp_t")
                        nc.sync.dma_start(
                            t[:], wp[pp_ * 128:(pp_ + 1) * 128, :])
                        wp_t.append(t)

                if pair == 0:
                    qk_group("k", 0, 0)
                    qk_group("q", 0, 0)
                for qb in range(QB):
                    hooks = {}
                    # host the NEXT block's Q-group at slot 13 so block
                    # boundaries carry no serial PE lump
                    bi = pair * QB + qb
                    if bi + 1 < PAIRS * QB:
                        npair, nqb = divmod(bi + 1, QB)
                        hooks.setdefault(13, []).append(
                            lambda p=npair, q=nqb: qk_group("q", p, q))
                    if pair == 0 and qb == 0:
                        # pair0: V chunks + remaining K-blocks just-in-time
                        # (last 4 V chunks spill into the next block's
                        # early slots: consumed there from slot 3 on)
                        for kc in range(KC):
                            if kc in (1, 5, 9):
                                kb = kc // 4 + 1
                                hooks.setdefault(kc, []).append(
                                    lambda kb=kb: qk_group("k", 0, kb))
                            if kc < 12:
                                hooks.setdefault(kc, []).append(
                                    lambda kc=kc: emit_v(0, kc))
                    else:
                        if pair == 0 and qb == 1:
                            for i, kc in enumerate(range(12, 16)):
                                hooks.setdefault(i // 2, []).append(
                                    lambda kc=kc: emit_v(0, kc))
                        # spread next pair's V (16 chunks) and K (4 groups)
                        # across slots 9..12 and 15 of qb1..3 blocks
                        if pair < PAIRS - 1 and qb >= 1:
                            np_ = pair + 1
                            hooks.setdefault(14, []).append(
                                lambda p=np_, kb=qb - 1:
                                qk_group("k", p, kb))
                            if qb == 3:
                                hooks.setdefault(15, []).append(
                                    lambda p=np_: qk_group("k", p, 3))
                                vcs = range(12, 16)
                            else:
                                vcs = range((qb - 1) * 6, qb * 6)
                            for i, kc in enumerate(vcs):
                                hooks.setdefault(11 + i // 2, []).append(
                                    lambda p=np_, kc=kc: emit_v(p, kc))
                        if pair == PAIRS - 1 and qb >= 1:
                            # stream prev q-block's proj as its per-chunk
                            # transposes (slots 6..9) land
                            slots = (7, 8, 9, 10, 11, 12, 13, 14)
                            groups = [((qb - 1) * 4 + qcw, nh)
                                      for qcw in range(4) for nh in range(2)]
                            for s, (c, n) in zip(slots, groups):
                                hooks.setdefault(s, []).append(
                                    lambda c=c, n=n:
                                    proj_group(c, n, defer=True))
                            hooks.setdefault(14, []).insert(
                                0, flush_out_dmas)
                    prev_st = block_kloop(pair, qb, prev_st, hooks)

            # --- tail: last block's PV software-pipelined with its proj ---
            tailpools = (None, s_pool)
            ti = 0

            def tail_proj(qcw):
                nonlocal ti
                qc = (QB - 1) * 4 + qcw
                for nh in range(2):
                    proj_group(qc, nh, pool=tailpools[ti % 2])
                    ti += 1

            pv_mms(prev_st, 0, (13, 14, 15))
            pv_norm(prev_st, 0)
            pv_mms(prev_st, 4, (13, 14, 15))
            pv_norm(prev_st, 4)
            pe_transpose(prev_st, 0)
            pv_region(prev_st, 1)
            pv_region(prev_st, 5)
            pe_transpose(prev_st, 1)
            flush_out_dmas()
            tail_proj(0)
            pv_region(prev_st, 2)
            pv_region(prev_st, 6)
            pe_transpose(prev_st, 2)
            tail_proj(1)
            pv_region(prev_st, 3)
            pv_region(prev_st, 7)
            pe_transpose(prev_st, 3)
            tail_proj(2)
            tail_proj(3)

            # safety net: any group not emitted above
            done = set(proj_done)
            for qc in range(QC):
                for nh in range(2):
                    if (qc, nh) not in done:
                        proj_group(qc, nh)


def build():
    if _NC_CACHE:
        return _NC_CACHE[0]
    nc = bacc.Bacc("TRN2", target_bir_lowering=False, debug=False,
                   enable_asserts=False, num_devices=N_CORES)
    xT = nc.dram_tensor("xT", [QB, 128, CC * 512], BF16,
                        kind="ExternalInput").ap()
    wq = nc.dram_tensor("wq", [PAIRS, 128, CC * 128], BF16,
                        kind="ExternalInput").ap()
    wk = nc.dram_tensor("wk", [PAIRS, 128, CC * 128], BF16,
                        kind="ExternalInput").ap()
    wv = nc.dram_tensor("wv", [128, CC * 512], BF16,
                        kind="ExternalInput").ap()
    mT = nc.dram_tensor("mT", [QB, KC // 4, 128, 4 * 512], BF16,
                        kind="ExternalInput").ap()
    wp = nc.dram_tensor("wp", [GW, C], BF16, kind="ExternalInput").ap()
    out = nc.dram_tensor("out", [N, C], BF16, kind="ExternalOutput").ap()
    with tile.TileContext(nc) as tc:
        _emit(tc, xT, wq, wk, wv, mT, wp, out)
    nc.compile()
    _NC_CACHE.append(nc)
    return nc


def _tile4(a, rows, cols):
    """[R, Q] -> [Q//cols, R//rows, rows, cols] contiguous tiles so every
    device DMA is a single contiguous transfer."""
    R, Q = a.shape
    return np.ascontiguousarray(
        a.reshape(R // rows, rows, Q // cols, cols).transpose(0, 2, 1, 3)
         .transpose(1, 0, 2, 3))


def _pack_cc(a, cols):
    """[C, Q] -> [Q//cols, 128, (C//128)*cols]: per q-block, the 8
    contraction chunks side by side on 128 partitions (one contiguous DMA
    per q-block)."""
    R, Q = a.shape
    t = a.reshape(R // 128, 128, Q // cols, cols)      # [cc, p, qb, c]
    return np.ascontiguousarray(
        t.transpose(2, 1, 0, 3).reshape(Q // cols, 128, (R // 128) * cols))


def shard_inputs(joint_feature, mask, W_qkv, W_proj, b_proj):
    mT = _tile4(np.ascontiguousarray(mask[0, 0].T).astype(ml_dtypes.bfloat16),
                128, 512)
    mT = np.ascontiguousarray(
        mT.reshape(QB, KC // 4, 4, 128, 512).transpose(0, 1, 3, 2, 4)
          .reshape(QB, KC // 4, 128, 4 * 512))
    in_maps = []
    for c in range(N_CORES):
        b, g = divmod(c, 2)
        lo, hi = g * GW, (g + 1) * GW
        in_maps.append({
            "xT": _pack_cc(np.ascontiguousarray(joint_feature[b].T)
                           .astype(ml_dtypes.bfloat16), 512),
            "wq": _pack_cc((W_qkv[:, lo:hi] * SCALE)
                           .astype(ml_dtypes.bfloat16), 128),
            "wk": _pack_cc(W_qkv[:, C + lo:C + hi]
                           .astype(ml_dtypes.bfloat16), 128),
            "wv": _pack_cc(W_qkv[:, 2 * C + lo:2 * C + hi]
                           .astype(ml_dtypes.bfloat16), 512)[0],
            "mT": mT,
            "wp": np.ascontiguousarray(W_proj[lo:hi, :])
                  .astype(ml_dtypes.bfloat16),
        })
    return in_maps


def kernel(joint_feature, mask, W_qkv, W_proj, b_proj):
    joint_feature = np.asarray(joint_feature, dtype=np.float32)
    mask = np.asarray(mask)
    W_qkv = np.asarray(W_qkv, dtype=np.float32)
    W_proj = np.asarray(W_proj, dtype=np.float32)
    b_proj = np.asarray(b_proj, dtype=np.float32)

    nc = build()
    in_maps = shard_inputs(joint_feature, mask, W_qkv, W_proj, b_proj)
    res = bass_utils.run_bass_kernel_spmd(nc, in_maps,
                                          core_ids=list(range(N_CORES)))
    out = np.empty((B, N, C), dtype=np.float32)
    for b in range(B):
        out[b] = np.asarray(res.results[2 * b]["out"], dtype=np.float32) \
            + np.asarray(res.results[2 * b + 1]["out"], dtype=np.float32) \
            + b_proj
    return out


# revision 63
# speedup vs baseline: 1.0898x; 1.0056x over previous
"""Bass/Trainium2 kernel for nn_Attention_46566035423948.

Multi-head attention (B=4, N=2048, C=1024, H=16) on 8 NeuronCores.
Sharding: core c = (batch b = c//2, head-group g = c%2, 8 heads each).
Each core computes a partial projection output [N, C] in bf16; the host
sums the two head-group partials per batch and adds b_proj.

Per-core dataflow (key-on-partition for S/exp, query-on-partition for PV):
  phase 1: Q^T,K^T [512, 2048] bf16 (head pairs packed 64+64 in partition
           chunks) and per-pair V [128, 2*65] k-chunk tiles (ones column
           per head; SCALE pre-folded into Wq on host) from bf16 xT and
           W_qkv slices. V/K/Q production for later blocks is spread
           across earlier blocks' k-loop slots so no block overloads PE.
  phase 2: per block = (head-pair, q-block 512), k-loop over 16 k-chunks:
           S^T = K^T.T @ Q^T (row-tiled pair of matmuls, emitted first in
           each slot so the ScalarE exp stream - the global pacer - never
           queues behind hook work on the in-order PE),
           P = exp(S^T) * mask (ScalarE exp PSUM->SBUF bf16, one DVE mul
           in place, mask broadcast across the two heads; mask DMAs are
           batched 4 chunks per transfer and loaded just in time).
           All 16 P tiles of a block are kept alive; the block's PV runs
           region-by-region streamed into the NEXT block's k-loop slots
           3..10: per (head, q-chunk 128) region, 16 consecutive matmuls
           with P as the stationary operand and V_aug (65 cols incl.
           ones) moving: U[q, 65] += P_chunk.T @ V_aug. Full 128 output
           partitions and 65 streamed rows per matmul (vs 512 with V
           stationary); col 64 accumulates the softmax denominator.
           Each region's matmuls are consecutive because PSUM allows one
           open accumulation group per bank. Regions start at slot 3
           because the boundary P tiles land ~2 slots into the block.
           Normalize: per-partition reciprocal of U[:,64] +
           tensor_scalar_mul into xn_q [128 q, 512] bf16. Transpose to
           xnT [128 dims, N]: pairs 0-2 use a whole-tile DMA-xbar
           transpose (free, slack of a full block); the last pair uses
           per-q-chunk PE transposes (identity matmul, ~0.6us chain) so
           its proj groups can stream into the same k-loop.
  phase 3: out = xnT.T @ W_proj_slice (bf16, accumulated over pairs in
           PSUM), staged to SBUF (ScalarE/DVE split) and written with
           output DMAs batched per (q-block, C-half, qc-pair); in the
           last pair's k-loops the DMA emission is deferred so its
           copy-wait cannot park on SP ahead of the transposes. The last
           block's first two PV regions pre-run inside its own k-loop
           (PSUM banks free after slot 10) and its tail interleaves the
           remaining regions, PE transposes and proj groups.
"""
import numpy as np
import ml_dtypes

import concourse.mybir as mybir
import concourse.tile as tile
from concourse import bacc
from concourse import bass_utils
from concourse.masks import make_identity

N_CORES = 8
B, N, C, H = 4, 2048, 1024, 16
HS = C // H           # 64
SCALE = HS ** -0.5
HPC = 8               # heads per core
GW = HPC * HS         # 512: per-core head-group width
PAIRS = 4             # head pairs per core
CC = C // 128         # 8 contraction chunks over C
KC = N // 128         # 16 key chunks
QB = N // 512         # 4 query blocks of 512
QC = N // 128         # 16 query chunks of 128 (proj)

F32 = mybir.dt.float32
BF16 = mybir.dt.bfloat16
EXP = mybir.ActivationFunctionType.Exp

_NC_CACHE = []


def _emit(tc, xT, wq, wk, wv, mT, wp, out):
    nc = tc.nc
    from contextlib import ExitStack

    with ExitStack() as stack:
        # persistent pools: V lives through phase 2, xnT through phase 3
        v_pool = stack.enter_context(tc.tile_pool(name="vp", bufs=PAIRS * KC))
        xn_pool = stack.enter_context(tc.tile_pool(name="xn", bufs=1))
        wp_pool = stack.enter_context(tc.tile_pool(name="wpp", bufs=PAIRS))
        ostage_pool = stack.enter_context(tc.tile_pool(name="ostage", bufs=8))

        v_t = {}
        xnT = [xn_pool.tile([128, N], BF16, name=f"xn{i}", tag=f"xn{i}")
               for i in range(PAIRS)]
        ident = xn_pool.tile([128, 128], BF16, name="ident", tag="ident")
        wp_t = []

        with tc.tile_pool(name="qkt", bufs=4) as qkt_pool, \
             tc.tile_pool(name="ep", bufs=28) as e_pool, \
             tc.tile_pool(name="rinv", bufs=4) as rinv_pool, \
             tc.tile_pool(name="xnq", bufs=2) as xnq_pool, \
             tc.tile_pool(name="xt", bufs=1) as xt_pool, \
             tc.tile_pool(name="wqk", bufs=8) as wqk_pool, \
             tc.tile_pool(name="wvp", bufs=1) as wv_pool, \
             tc.tile_pool(name="mp", bufs=3) as m_pool, \
             tc.tile_pool(name="ps2s", bufs=2, space="PSUM") as s_pool, \
             tc.tile_pool(name="ps1", bufs=2, space="PSUM") as ps1_pool, \
             tc.tile_pool(name="ps2u", bufs=1, space="PSUM") as u_pool:

            make_identity(nc, ident[:])

            # warm the PE clock during the input-DMA wait: the p-state
            # model halves matmul throughput until ~3us of sustained
            # execution, which would otherwise tax the first QK groups
            warm = xnq_pool.tile([128, 512], BF16, name="warm", tag="xn_q")
            nc.gpsimd.memset(warm[:], 0.0)
            wps = ps1_pool.tile([128, 512], F32, name="ps1t", tag="ps1t")
            for i in range(8):
                nc.tensor.matmul(wps[:], warm[:, 0:128], warm[:],
                                 start=(i == 0), stop=(i == 7))

            # --- input DMA, in PE-consumption order ----------------------
            wts = {}           # (which, pair, cc) -> weight slice

            def dma_w(which, wsrc, pair):
                wt = wqk_pool.tile([128, CC * 128], BF16, name="wqk_t",
                                   tag="wqk_t")
                nc.sync.dma_start(wt[:], wsrc[pair])
                for cc in range(CC):
                    wts[(which, pair, cc)] = wt[:, cc * 128:(cc + 1) * 128]

            dma_w("k", wk, 0)
            xt_q = {}
            wv_t = []
            t = xt_pool.tile([128, CC * 512], BF16, name="xt_0")
            half = CC * 256
            nc.sync.dma_start(t[:, 0:half], xT[0, :, 0:half])
            nc.sync.dma_start(t[:, half:], xT[0, :, half:])
            for cc in range(CC):
                xt_q[(cc, 0)] = t[:, cc * 512:(cc + 1) * 512]
            dma_w("q", wq, 0)
            t = wv_pool.tile([128, CC * 512], BF16, name="wv_all")
            nc.sync.dma_start(t[:, 0:half], wv[:, 0:half])
            nc.sync.dma_start(t[:, half:], wv[:, half:])
            for cc in range(CC):
                wv_t.append(t[:, cc * 512:(cc + 1) * 512])
            for qb in range(1, QB):
                t = xt_pool.tile([128, CC * 512], BF16, name=f"xt_{qb}")
                nc.sync.dma_start(t[:], xT[qb])
                for cc in range(CC):
                    xt_q[(cc, qb)] = t[:, cc * 512:(cc + 1) * 512]
            for p in range(1, PAIRS):
                dma_w("k", wk, p)
                dma_w("q", wq, p)

            # --- building blocks -----------------------------------------
            def emit_v(pair, kc):
                """V for (pair, k-chunk): [128 pos, 2*65] bf16 with ones
                columns (PV moving operand)."""
                ps = ps1_pool.tile([128, 128], F32, name="psv", tag="ps1t")
                for cc in range(CC):
                    nc.tensor.matmul(
                        ps[:],
                        xt_q[(cc, kc // 4)][:, (kc % 4) * 128:
                                            (kc % 4) * 128 + 128],
                        wv_t[cc][:, pair * 128:(pair + 1) * 128],
                        start=(cc == 0), stop=(cc == CC - 1))
                t = v_pool.tile([128, 130], BF16, name="v_t", tag="v_t")
                tv = t[:].rearrange("p (h d) -> p h d", h=2)
                nc.gpsimd.memset(tv[:, :, HS:HS + 1], 1.0)
                nc.vector.tensor_copy(
                    tv[:, :, 0:HS],
                    ps[:].rearrange("p (h d) -> p h d", h=2))
                v_t[(pair, kc)] = t

            qkt = {}

            def qk_group(which, pair, qb, cols=None):
                dst = qkt[(which, pair)]
                c0, c1 = (0, 512) if cols is None else cols
                w = c1 - c0
                ps = ps1_pool.tile([128, 512], F32, name="ps1t", tag="ps1t")
                for cc in range(CC):
                    nc.tensor.matmul(
                        ps[:, 0:w], wts[(which, pair, cc)][:],
                        xt_q[(cc, qb)][:, c0:c1],
                        start=(cc == 0), stop=(cc == CC - 1))
                nc.vector.tensor_copy(
                    dst[:, qb * 512 + c0:qb * 512 + c1], ps[:, 0:w])

            proj_done = []
            ost_tiles = {}     # (qb, nh, half) -> (tile, count)
            deferred_dmas = []

            def flush_out_dmas():
                while deferred_dmas:
                    dst, ost = deferred_dmas.pop(0)
                    nc.sync.dma_start(dst, ost)

            def proj_group(qc, nh, pool=None, defer=False):
                if pool is None:
                    ps = ps1_pool.tile([128, 512], F32, name="ps1t",
                                       tag="ps1t")
                else:
                    # tail: rotate through the (now idle) S ring so proj
                    # groups pipeline 4 deep without extra PSUM
                    ps = pool.tile([128, 512], F32, name="S", tag="S")
                for pair_ in range(PAIRS):
                    nc.tensor.matmul(
                        ps[:], xnT[pair_][:, qc * 128:(qc + 1) * 128],
                        wp_t[pair_][:, nh * 512:(nh + 1) * 512],
                        start=(pair_ == 0), stop=(pair_ == PAIRS - 1))
                qb_, qcw_ = divmod(qc, 4)
                key = (qb_, nh, qcw_ // 2)
                if key not in ost_tiles:
                    ost_tiles[key] = [ostage_pool.tile(
                        [128, 1024], BF16, name="ost", tag="ost"), 0]
                ost, cnt = ost_tiles[key]
                if nh == 0:
                    nc.scalar.copy(
                        ost[:, (qcw_ % 2) * 512:(qcw_ % 2 + 1) * 512], ps[:])
                else:
                    nc.vector.tensor_copy(
                        ost[:, (qcw_ % 2) * 512:(qcw_ % 2 + 1) * 512], ps[:])
                ost_tiles[key][1] = cnt + 1
                if cnt + 1 == 2:
                    # one batched output DMA per (q-block, C-half, qc-pair);
                    # in the last pair's k-loops the DMA emission is
                    # deferred so its copy-wait can't park on SP ahead of
                    # the next block's transposes
                    r0 = qb_ * 512 + (qcw_ // 2) * 256
                    dst = out[r0:r0 + 256, nh * 512:(nh + 1) * 512] \
                        .rearrange("(qc p) q -> p qc q", p=128)
                    srcv = ost[:].rearrange("p (qc q) -> p qc q", qc=2)
                    if defer:
                        deferred_dmas.append((dst, srcv))
                    else:
                        nc.sync.dma_start(dst, srcv)
                proj_done.append((qc, nh))

            # --- deferred PV machinery ------------------------------------
            def pv_mms(st, r, kcs):
                """Accumulation matmuls for one (head, q-chunk) region.
                All of a region's matmuls form one PSUM group (one open
                group per bank), but may be emitted in separate runs."""
                pair, qb, p_tiles, U, xn_q = st
                head, qcw = divmod(r, 4)
                for kc in kcs:
                    nc.tensor.matmul(
                        U[:, r * 128:r * 128 + 65],
                        p_tiles[kc][:, head * 512 + qcw * 128:
                                    head * 512 + qcw * 128 + 128],
                        v_t[(pair, kc)][:, head * 65:head * 65 + 65],
                        start=(kc == 0), stop=(kc == KC - 1),
                        tile_position=(0, 0), skip_group_check=True)

            def pv_norm(st, r):
                pair, qb, p_tiles, U, xn_q = st
                head, qcw = divmod(r, 4)
                rinv = rinv_pool.tile([128, 1], F32, name="rinv", tag="rinv")
                nc.vector.reciprocal(rinv[:], U[:, r * 128 + 64:r * 128 + 65])
                nc.vector.tensor_scalar_mul(
                    xn_q[:, qcw * 128 + head * 64:qcw * 128 + head * 64 + 64],
                    U[:, r * 128:r * 128 + 64], rinv[:])

            def pv_region(st, r):
                pv_mms(st, r, range(KC))
                pv_norm(st, r)

            def pe_transpose(st, qcw):
                """Last-pair transpose via the PE array (PSUM-local, ~0.6us
                chain) instead of the ~3us DMA-xbar path."""
                pair, qb, p_tiles, U, xn_q = st
                pt = ps1_pool.tile([128, 128], BF16, name="ptt", tag="ps1t")
                nc.tensor.transpose(
                    pt[:], xn_q[:, qcw * 128:(qcw + 1) * 128], ident[:])
                nc.vector.tensor_copy(
                    xnT[pair][:, qb * 512 + qcw * 128:
                              qb * 512 + (qcw + 1) * 128], pt[:])

            def pv_transpose(st, qcw=None):
                pair, qb, p_tiles, U, xn_q = st
                if qcw is None:
                    nc.sync.dma_start_transpose(
                        xnT[pair][:, qb * 512:(qb + 1) * 512]
                        .rearrange("p (four q) -> p four q", four=4),
                        xn_q[:])
                else:
                    nc.sync.dma_start_transpose(
                        xnT[pair][:, qb * 512 + qcw * 128:
                                  qb * 512 + (qcw + 1) * 128],
                        xn_q[:, qcw * 128:(qcw + 1) * 128])

            def block_kloop(pair, qb, prev_st, slot_hooks):
                """S/exp/mask k-loop for one block; streams prev block's PV
                regions (slots 3..10) and its transposes."""
                U = u_pool.tile([128, 1024], F32, name="U", tag="U")
                xn_q = xnq_pool.tile([128, 512], BF16, name="xn_q",
                                     tag="xn_q")
                qs = slice(qb * 512, (qb + 1) * 512)
                KT = qkt[("k", pair)]
                QT = qkt[("q", pair)]
                mtiles = []

                def load_mask(g):
                    mt = m_pool.tile([128, 2048], BF16, name="m_t",
                                     tag="m_t")
                    nc.sync.dma_start(mt[:], mT[qb, g])
                    for i in range(4):
                        mtiles.append(mt[:, i * 512:(i + 1) * 512])

                load_mask(0)
                load_mask(1)
                p_tiles = []
                for kc in range(KC):
                    if kc == 2:
                        load_mask(2)
                    elif kc == 6:
                        load_mask(3)
                    # S + exp + mask first: the exp stream is the global
                    # pacer, so the next S must never queue behind a slot's
                    # hook lump on the in-order PE
                    ks = slice(kc * 128, (kc + 1) * 128)
                    S = s_pool.tile([128, 1024], F32, name="S", tag="S")
                    nc.tensor.matmul(S[:, 0:512], KT[0:64, ks], QT[0:64, qs],
                                     start=True, stop=True,
                                     tile_position=(0, 0))
                    nc.tensor.matmul(S[:, 512:1024], KT[64:128, ks],
                                     QT[64:128, qs],
                                     start=True, stop=True,
                                     tile_position=(64, 0))
                    E = e_pool.tile([128, 1024], BF16, name="E", tag="E")
                    nc.scalar.activation(E[:], S[:], EXP)
                    ev = E[:].rearrange("p (two q) -> p two q", two=2)
                    # last pair: DVE also carries the proj staging copies,
                    # so shift a quarter of the mask-mults to idle GpSimd
                    eng = nc.gpsimd if pair == PAIRS - 1 and kc % 8 == 0 \
                        else nc.vector
                    eng.tensor_mul(
                        ev, ev, mtiles[kc][:, None, :]
                        .broadcast_to((128, 2, 512)))
                    p_tiles.append(E)
                    if prev_st is not None:
                        # regions start at slot 3: P(kc15) of the previous
                        # block is only ready ~2 slots after the boundary,
                        # and a region's final accumulation matmul would
                        # park the in-order PE queue until it lands.
                        # Paired order (a q-chunk's two heads adjacent) so
                        # each q-chunk's transpose can issue early.
                        if 3 <= kc <= 10:
                            qcw_, h_ = divmod(kc - 3, 2)
                            pv_region(prev_st, h_ * 4 + qcw_)
                            if h_ == 1 and prev_st[0] == PAIRS - 1:
                                # prev block is in the last pair: immediate
                                # per-q-chunk PE transpose so its proj can
                                # stream this block with a short chain
                                pe_transpose(prev_st, qcw_)
                        if prev_st[0] != PAIRS - 1 and kc == 11:
                            pv_transpose(prev_st)
                    for fn in slot_hooks.get(kc, ()):
                        fn()
                    if pair == PAIRS - 1 and qb == QB - 1 and kc >= 12:
                        # pre-run the tail's first two regions (banks 0/1)
                        # against the P tiles already produced; PSUM is
                        # free of the prev block's U after slot 10
                        st_self = (pair, qb, p_tiles, U, xn_q)
                        if kc == 12:
                            pv_mms(st_self, 0, range(0, 10))
                        elif kc == 13:
                            pv_mms(st_self, 4, range(0, 10))
                        elif kc == 14:
                            pv_mms(st_self, 0, (10, 11))
                            pv_mms(st_self, 4, (10, 11))
                        else:
                            pv_mms(st_self, 0, (12,))
                            pv_mms(st_self, 4, (12,))
                return (pair, qb, p_tiles, U, xn_q)

            # --- drive the 16 blocks --------------------------------------
            prev_st = None
            qkt[("k", 0)] = qkt_pool.tile([128, N], BF16, name="qkt_t",
                                          tag="qkt_t")
            qkt[("q", 0)] = qkt_pool.tile([128, N], BF16, name="qkt_t",
                                          tag="qkt_t")
            for pair in range(PAIRS):
                if pair < PAIRS - 1:
                    qkt[("k", pair + 1)] = qkt_pool.tile(
                        [128, N], BF16, name="qkt_t", tag="qkt_t")
                    qkt[("q", pair + 1)] = qkt_pool.tile(
                        [128, N], BF16, name="qkt_t", tag="qkt_t")
                if pair == 1:
                    # prefetch proj weights once SBUF headroom exists
                    for pp_ in range(PAIRS):
                        t = wp_pool.tile([128, C], BF16, name="wp_t",
                                         tag="wp_t")
                        nc.sync.dma_start(
                            t[:], wp[pp_ * 128:(pp_ + 1) * 128, :])
                        wp_t.append(t)

                if pair == 0:
                    qk_group("k", 0, 0)
                    qk_group("q", 0, 0)
                for qb in range(QB):
                    hooks = {}
                    # host the NEXT block's Q-group at slot 13 so block
                    # boundaries carry no serial PE lump
                    bi = pair * QB + qb
                    if bi + 1 < PAIRS * QB:
                        npair, nqb = divmod(bi + 1, QB)
                        hooks.setdefault(13, []).append(
                            lambda p=npair, q=nqb: qk_group("q", p, q))
                    if pair == 0 and qb == 0:
                        # pair0: V chunks + remaining K-blocks just-in-time
                        # (last 4 V chunks spill into the next block's
                        # early slots: consumed there from slot 3 on)
                        for kc in range(KC):
                            if kc in (1, 5, 9):
                                kb = kc // 4 + 1
                                hooks.setdefault(kc, []).append(
                                    lambda kb=kb: qk_group("k", 0, kb))
                            if kc < 12:
                                hooks.setdefault(kc, []).append(
                                    lambda kc=kc: emit_v(0, kc))
                    else:
                        if pair == 0 and qb == 1:
                            for i, kc in enumerate(range(12, 16)):
                                hooks.setdefault(i // 2, []).append(
                                    lambda kc=kc: emit_v(0, kc))
                        # spread next pair's V (16 chunks) and K (4 groups)
                        # across slots 9..12 and 15 of qb1..3 blocks
                        if pair < PAIRS - 1 and qb >= 1:
                            np_ = pair + 1
                            hooks.setdefault(14, []).append(
                                lambda p=np_, kb=qb - 1:
                                qk_group("k", p, kb))
                            if qb == 3:
                                hooks.setdefault(15, []).append(
                                    lambda p=np_: qk_group("k", p, 3))
                                vcs = range(12, 16)
                            else:
                                vcs = range((qb - 1) * 6, qb * 6)
                            for i, kc in enumerate(vcs):
                                hooks.setdefault(13 + i // 2, []).append(
                                    lambda p=np_, kc=kc: emit_v(p, kc))
                        if pair == PAIRS - 1 and qb >= 1:
                            # stream prev q-block's proj as its per-chunk
                            # transposes (slots 6..9) land
                            slots = (7, 8, 9, 10, 11, 12, 13, 14)
                            groups = [((qb - 1) * 4 + qcw, nh)
                                      for qcw in range(4) for nh in range(2)]
                            for s, (c, n) in zip(slots, groups):
                                hooks.setdefault(s, []).append(
                                    lambda c=c, n=n:
                                    proj_group(c, n, defer=True))
                            hooks.setdefault(14, []).insert(
                                0, flush_out_dmas)
                    prev_st = block_kloop(pair, qb, prev_st, hooks)

            # --- tail: last block's PV software-pipelined with its proj ---
            tailpools = (None, s_pool)
            ti = 0

            def tail_proj(qcw):
                nonlocal ti
                qc = (QB - 1) * 4 + qcw
                for nh in range(2):
                    proj_group(qc, nh, pool=tailpools[ti % 2])
                    ti += 1

            pv_mms(prev_st, 0, (13, 14, 15))
            pv_norm(prev_st, 0)
            pv_mms(prev_st, 4, (13, 14, 15))
            pv_norm(prev_st, 4)
            pe_transpose(prev_st, 0)
            pv_region(prev_st, 1)
            pv_region(prev_st, 5)
            pe_transpose(prev_st, 1)
            flush_out_dmas()
            tail_proj(0)
            pv_region(prev_st, 2)
            pv_region(prev_st, 6)
            pe_transpose(prev_st, 2)
            tail_proj(1)
            pv_region(prev_st, 3)
            pv_region(prev_st, 7)
            pe_transpose(prev_st, 3)
            tail_proj(2)
            tail_proj(3)

            # safety net: any group not emitted above
            done = set(proj_done)
            for qc in range(QC):
                for nh in range(2):
                    if (qc, nh) not in done:
                        proj_group(qc, nh)


def build():
    if _NC_CACHE:
        return _NC_CACHE[0]
    nc = bacc.Bacc("TRN2", target_bir_lowering=False, debug=False,
                   enable_asserts=False, num_devices=N_CORES)
    xT = nc.dram_tensor("xT", [QB, 128, CC * 512], BF16,
                        kind="ExternalInput").ap()
    wq = nc.dram_tensor("wq", [PAIRS, 128, CC * 128], BF16,
                        kind="ExternalInput").ap()
    wk = nc.dram_tensor("wk", [PAIRS, 128, CC * 128], BF16,
                        kind="ExternalInput").ap()
    wv = nc.dram_tensor("wv", [128, CC * 512], BF16,
                        kind="ExternalInput").ap()
    mT = nc.dram_tensor("mT", [QB, KC // 4, 128, 4 * 512], BF16,
                        kind="ExternalInput").ap()
    wp = nc.dram_tensor("wp", [GW, C], BF16, kind="ExternalInput").ap()
    out = nc.dram_tensor("out", [N, C], BF16, kind="ExternalOutput").ap()
    with tile.TileContext(nc) as tc:
        _emit(tc, xT, wq, wk, wv, mT, wp, out)
    nc.compile()
    _NC_CACHE.append(nc)
    return nc


def _tile4(a, rows, cols):
    """[R, Q] -> [Q//cols, R//rows, rows, cols] contiguous tiles so every
    device DMA is a single contiguous transfer."""
    R, Q = a.shape
    return np.ascontiguousarray(
        a.reshape(R // rows, rows, Q // cols, cols).transpose(0, 2, 1, 3)
         .transpose(1, 0, 2, 3))


def _pack_cc(a, cols):
    """[C, Q] -> [Q//cols, 128, (C//128)*cols]: per q-block, the 8
    contraction chunks side by side on 128 partitions (one contiguous DMA
    per q-block)."""
    R, Q = a.shape
    t = a.reshape(R // 128, 128, Q // cols, cols)      # [cc, p, qb, c]
    return np.ascontiguousarray(
        t.transpose(2, 1, 0, 3).reshape(Q // cols, 128, (R // 128) * cols))


def shard_inputs(joint_feature, mask, W_qkv, W_proj, b_proj):
    mT = _tile4(np.ascontiguousarray(mask[0, 0].T).astype(ml_dtypes.bfloat16),
                128, 512)
    mT = np.ascontiguousarray(
        mT.reshape(QB, KC // 4, 4, 128, 512).transpose(0, 1, 3, 2, 4)
          .reshape(QB, KC // 4, 128, 4 * 512))
    in_maps = []
    for c in range(N_CORES):
        b, g = divmod(c, 2)
        lo, hi = g * GW, (g + 1) * GW
        in_maps.append({
            "xT": _pack_cc(np.ascontiguousarray(joint_feature[b].T)
                           .astype(ml_dtypes.bfloat16), 512),
            "wq": _pack_cc((W_qkv[:, lo:hi] * SCALE)
                           .astype(ml_dtypes.bfloat16), 128),
            "wk": _pack_cc(W_qkv[:, C + lo:C + hi]
                           .astype(ml_dtypes.bfloat16), 128),
            "wv": _pack_cc(W_qkv[:, 2 * C + lo:2 * C + hi]
                           .astype(ml_dtypes.bfloat16), 512)[0],
            "mT": mT,
            "wp": np.ascontiguousarray(W_proj[lo:hi, :])
                  .astype(ml_dtypes.bfloat16),
        })
    return in_maps


def kernel(joint_feature, mask, W_qkv, W_proj, b_proj):
    joint_feature = np.asarray(joint_feature, dtype=np.float32)
    mask = np.asarray(mask)
    W_qkv = np.asarray(W_qkv, dtype=np.float32)
    W_proj = np.asarray(W_proj, dtype=np.float32)
    b_proj = np.asarray(b_proj, dtype=np.float32)

    nc = build()
    in_maps = shard_inputs(joint_feature, mask, W_qkv, W_proj, b_proj)
    res = bass_utils.run_bass_kernel_spmd(nc, in_maps,
                                          core_ids=list(range(N_CORES)))
    out = np.empty((B, N, C), dtype=np.float32)
    for b in range(B):
        out[b] = np.asarray(res.results[2 * b]["out"], dtype=np.float32) \
            + np.asarray(res.results[2 * b + 1]["out"], dtype=np.float32) \
            + b_proj
    return out
